# revision 12
# baseline (speedup 1.0000x reference)
"""Distributed attention kernel for 8 TRN2 NeuronCores.

Problem: L=2048, B=2, E=256, H=8 heads, D=32 head-dim, fp32.

Sharding: DP2 over batch x sequence-parallel-4 over query positions.
Core c handles batch c//4, query rows [512*(c%4), 512*(c%4+1)), ALL 8
heads. k/v projections are redundantly computed per batch group (cheap)
and NO collective is needed: each core owns a disjoint output block.

Key structure (v2):
  - Softmax exp is split across TWO engines: ScalarE runs exact table
    exp (with fused 1/sqrt(D) scale); VectorE runs a Schraudolph
    bit-trick exp (i16 = x*A + B, reinterpreted as bf16 bits) for a
    subset of chunks, calibrated for zero softmax-weighted bias.
  - PV uses v as the STATIONARY operand: per (head, tk-chunk) one
    [128]-contraction matmul streaming 512 q columns into a [33, 512]
    psum block accumulated across all 16 chunks. The ones column of
    each [v|1] slot yields the softmax denominator Z as psum row 32.
  - Normalization: rz = 1/Z (DVE reciprocal straight from psum row),
    broadcast to 32 partitions with a K=1 fp32r ones-matmul into the
    same psum bank (partitions 64:96), then one scalar_tensor_tensor
    per head writes normalized O^T directly into the projection's
    lhsT layout. No DMA transposes anywhere.
  - Projection accumulates per 4-head group; biases are exact.
"""

import os
import sys

import numpy as np

for _p in ("/opt/trn_rl_repo",):
    if _p not in sys.path and os.path.isdir(_p):
        sys.path.insert(0, _p)

import ml_dtypes

import concourse.bass as bass
import concourse.bacc as bacc
import concourse.mybir as mybir
import concourse.tile as tile
from concourse.bass_utils import run_bass_kernel_spmd

dt = mybir.dt
F32 = dt.float32
F32R = dt.float32r
BF16 = dt.bfloat16
I16 = dt.int16
AF = mybir.ActivationFunctionType
ALU = mybir.AluOpType
BF = ml_dtypes.bfloat16

L, B, E, H, D = 2048, 2, 256, 8, 32
SCALE = float(D) ** -0.5
NCORES = 8
SP = 4            # sequence-parallel ways
TQ = L // SP      # 512 query rows per core
NTK = L // 128    # 16 tk chunks
VW = H * (D + 1)  # v_buf cols per tk chunk: 8x [v_h | 1] = 264

# Schraudolph exp constants (bf16-bits variant): i16 = round(s*A + B)
# viewed as bf16 equals exp(s*SCALE) with ~1.8% rms ripple and zero
# softmax-weighted mean (C calibrated numerically).
A_SCH = SCALE * (2.0 ** 23 / np.log(2.0)) / 65536.0
B_SCH = 127.0 * 128.0 - 483000.0 / 65536.0

# Of the 64 (head, chunk-pair) exp tiles, this many go to VectorE
# (Schraudolph); the rest run exact exp on ScalarE.
N_DVE = int(os.environ.get('N_DVE', 19))
DVE_CHUNKS = {(i * 64) // N_DVE for i in range(N_DVE)}

_GRAPH = None


def _build_graph():
    nc = bacc.Bacc(
        "TRN2",
        target_bir_lowering=False,
        debug=False,
        enable_asserts=False,
        num_devices=NCORES,
    )

    xqt = nc.declare_dram_parameter("xqt", [E, TQ], BF16, isOutput=False).ap()
    xkt = nc.declare_dram_parameter("xkt", [E, L], BF16, isOutput=False).ap()
    xvt = nc.declare_dram_parameter("xvt", [E, L], BF16, isOutput=False).ap()
    wq = nc.declare_dram_parameter("wq", [E, E], BF16, isOutput=False).ap()
    wk = nc.declare_dram_parameter("wk", [E, E], BF16, isOutput=False).ap()
    wv = nc.declare_dram_parameter("wv", [E, E], BF16, isOutput=False).ap()
    wp = nc.declare_dram_parameter("wp", [E, E], BF16, isOutput=False).ap()
    bq = nc.declare_dram_parameter("bq", [1, E], F32, isOutput=False).ap()
    bk = nc.declare_dram_parameter("bk", [1, E], F32, isOutput=False).ap()
    bv = nc.declare_dram_parameter("bv", [1, E], F32, isOutput=False).ap()
    bp = nc.declare_dram_parameter("bp", [1, E], F32, isOutput=False).ap()
    out = nc.declare_dram_parameter("out", [TQ, E], F32, isOutput=True).ap()

    with tile.TileContext(nc) as tc:
        with (
            tc.tile_pool(name="persist", bufs=1) as pp,
            tc.tile_pool(name="pt", bufs=4) as ptp,
            tc.tile_pool(name="oT", bufs=2) as oTp,
            tc.tile_pool(name="rz", bufs=4) as rzp,
            tc.tile_pool(name="zb", bufs=2) as zbp,
            tc.tile_pool(name="outsb", bufs=4) as outp,
            tc.tile_pool(name="st", bufs=2, space="PSUM") as stp,
            tc.tile_pool(name="pv", bufs=2, space="PSUM") as pvp,
            tc.tile_pool(name="pj", bufs=2, space="PSUM") as pjp,
        ):
            # ---------- phase 0: loads ----------
            # warm the Exp activation table before the first real exp
            warm = pp.tile([1, 16], F32)
            nc.vector.memset(warm[:], 0.0)
            nc.scalar.activation(warm[:], warm[:], AF.Exp)

            # weights: tile [128, 2E]; slice e covers W rows [128e, 128e+128)
            w_sb = {}

            def load_w(name, wsrc):
                t = pp.tile([128, 2 * E], BF16, name=f"w{name}", tag=f"w{name}")
                nc.scalar.dma_start(
                    out=t[:].rearrange("p (e n) -> p e n", e=2),
                    in_=wsrc.rearrange("(e p) n -> p e n", p=128),
                )
                w_sb[name] = t

            load_w("k", wk)
            load_w("q", wq)

            # biases: bq/bk as per-partition columns [128, 2] (hc chunks);
            # bv/bp replicated across partitions
            bq_sb = pp.tile([128, 2], F32)
            nc.gpsimd.dma_start(
                out=bq_sb[:], in_=bq.rearrange("a (c p) -> p (a c)", p=128)
            )
            bk_sb = pp.tile([128, 2], F32)
            nc.gpsimd.dma_start(
                out=bk_sb[:], in_=bk.rearrange("a (c p) -> p (a c)", p=128)
            )
            bv_sb = pp.tile([128, E], F32)
            nc.gpsimd.dma_start(out=bv_sb[:], in_=bv.to_broadcast((128, E)))
            bp_sb = pp.tile([128, E], F32)
            nc.gpsimd.dma_start(out=bp_sb[:], in_=bp.to_broadcast((128, E)))

            # x.T streams: xk on SyncE queue, xq on VectorE, xv on GpSimd
            xk_sb = [
                pp.tile([128, L], BF16, name=f"xkt{e}", tag=f"xkt{e}")
                for e in range(2)
            ]
            for n in range(2):
                for e in range(2):
                    nc.sync.dma_start(
                        out=xk_sb[e][:, n * 1024:(n + 1) * 1024],
                        in_=xkt[e * 128:(e + 1) * 128, n * 1024:(n + 1) * 1024],
                    )
            xq_sb = []
            for e in range(2):
                t = pp.tile([128, TQ], BF16, name=f"xqt{e}", tag=f"xqt{e}")
                nc.sync.dma_start(out=t[:], in_=xqt[e * 128:(e + 1) * 128, :])
                xq_sb.append(t)
            load_w("v", wv)
            load_w("p", wp)
            xv_sb = [
                pp.tile([128, L], BF16, name=f"xvt{e}", tag=f"xvt{e}")
                for e in range(2)
            ]
            for n in range(2):
                for e in range(2):
                    nc.gpsimd.dma_start(
                        out=xv_sb[e][:, n * 1024:(n + 1) * 1024],
                        in_=xvt[e * 128:(e + 1) * 128, n * 1024:(n + 1) * 1024],
                    )

            # v_buf: per tk chunk, 8x [v_h (32) | 1] slots; set only the
            # ones columns (strided memset, 128 elements)
            v_buf = pp.tile([128, NTK * VW], BF16)
            nc.gpsimd.memset(
                v_buf[:].rearrange("p (t h w) -> p t h w", t=NTK, h=H)[
                    :, :, :, D:D + 1
                ],
                1.0,
            )

            # ---------- phase 1: projections ----------
            # kT: [256 head-dims, 2048] as four [64, 2048] tiles
            # (2 heads per tile at partition bases 0/32)
            ncopy = 0

            def copy_bias(dst, src, bias_ap):
                # psum->sbuf copy + per-partition bias, alternating engines
                nonlocal ncopy
                if os.environ.get('ALL_DVE_COPY') or ncopy % 2 == 0:
                    nc.vector.tensor_scalar_add(dst, src, bias_ap)
                else:
                    nc.scalar.activation(dst, src, AF.Identity, bias=bias_ap)
                ncopy += 1

            kT = [pp.tile([64, L], BF16, name=f"kT{pc}", tag=f"kT{pc}")
                  for pc in range(4)]
            for hc in range(2):
                for n in range(L // 512):
                    ps = pjp.tile([128, 512], F32, tag="pj")
                    for e in range(2):
                        nc.tensor.matmul(
                            ps[:],
                            w_sb["k"][:, e * E + hc * 128: e * E + (hc + 1) * 128],
                            xk_sb[e][:, n * 512:(n + 1) * 512],
                            start=(e == 0),
                            stop=(e == 1),
                        )
                    for half in range(2):
                        copy_bias(
                            kT[2 * hc + half][:, n * 512:(n + 1) * 512],
                            ps[half * 64:(half + 1) * 64, :],
                            bk_sb[half * 64:(half + 1) * 64, hc:hc + 1],
                        )

            # q.T slice: four [64, 512] tiles
            qT = [pp.tile([64, TQ], BF16, name=f"qT{pc}", tag=f"qT{pc}")
                  for pc in range(4)]
            for hc in range(2):
                ps = pjp.tile([128, TQ], F32, tag="pj")
                for e in range(2):
                    nc.tensor.matmul(
                        ps[:],
                        w_sb["q"][:, e * E + hc * 128: e * E + (hc + 1) * 128],
                        xq_sb[e][:, :],
                        start=(e == 0),
                        stop=(e == 1),
                    )
                for half in range(2):
                    copy_bias(
                        qT[2 * hc + half][:, :],
                        ps[half * 64:(half + 1) * 64, :],
                        bq_sb[half * 64:(half + 1) * 64, hc:hc + 1],
                    )

            # v (natural layout): per tk chunk -> strided scatter into the
            # [v|1] slots fused with the bias add (one DVE op per chunk)
            for t in range(NTK):
                ps = pjp.tile([128, E], F32, tag="pj")
                for e in range(2):
                    nc.tensor.matmul(
                        ps[:],
                        xv_sb[e][:, t * 128:(t + 1) * 128],
                        w_sb["v"][:, e * E:(e + 1) * E],
                        start=(e == 0),
                        stop=(e == 1),
                    )
                nc.vector.tensor_tensor(
                    v_buf[:, t * VW:(t + 1) * VW].rearrange(
                        "p (h w) -> p h w", h=H
                    )[:, :, 0:D],
                    ps[:].rearrange("p (h w) -> p h w", h=H),
                    bv_sb[:].rearrange("p (h w) -> p h w", h=H),
                    ALU.add,
                )

            # proj psum: two [128, 512] tiles hold the four [128, 256]
            # tq-chunk partials, accumulated across both head groups
            pjt = [pjp.tile([128, 2 * E], F32, name=f"pjt{i}", tag="pj")
                   for i in range(2)]

            # ---------- phase 2: attention ----------
            chunk_idx = 0
            for grp in range(2):
                oTg = oTp.tile([128, TQ], BF16, tag="oT")
                for u in range(4):
                    h = grp * 4 + u
                    hc, hr = h // 2, (h % 2) * D
                    pv = pvp.tile([D + 1, TQ], F32, tag="pv")
                    for g in range(NTK // 2):
                        st = stp.tile([128, 1024], F32, tag="st")
                        for i in range(2):
                            tk = 2 * g + i
                            nc.tensor.matmul(
                                st[:, i * 512:(i + 1) * 512],
                                kT[hc][hr:hr + D, tk * 128:(tk + 1) * 128],
                                qT[hc][hr:hr + D, :],
                                start=True,
                                stop=True,
                            )
                        pt = ptp.tile([128, 1024], BF16, tag="pt")
                        if chunk_idx in DVE_CHUNKS:
                            nc.vector.tensor_scalar(
                                pt[:].bitcast(I16), st[:], A_SCH, B_SCH,
                                op0=ALU.mult, op1=ALU.add,
                            )
                        else:
                            nc.scalar.activation(
                                pt[:], st[:], AF.Exp, scale=SCALE
                            )
                        chunk_idx += 1
                        for i in range(2):
                            tk = 2 * g + i
                            nc.tensor.matmul(
                                pv[0:D + 1, :],
                                v_buf[:, tk * VW + h * (D + 1):
                                      tk * VW + (h + 1) * (D + 1)],
                                pt[:, i * 512:(i + 1) * 512],
                                start=(g == 0 and i == 0),
                                stop=(g == NTK // 2 - 1 and i == 1),
                                skip_group_check=True,
                            )

                    # normalize head h: rz = 1/Z from psum row 32 (DVE
                    # reciprocal straight from psum), partition-broadcast
                    # to [32, TQ] via a stride-0 SBUF->SBUF DMA (free), then
                    # one fused multiply writes O^T into the projection
                    # layout (only one PSUM operand per DVE op is legal)
                    rz = rzp.tile([1, TQ], F32, tag="rz")
                    nc.vector.reciprocal(rz[:], pv[D:D + 1, :])
                    zb = zbp.tile([D, TQ], F32, tag="zb")
                    nc.gpsimd.partition_broadcast(zb[:], rz[:])
                    nc.vector.scalar_tensor_tensor(
                        oTg[u * D:(u + 1) * D, :],
                        pv[0:D, :],
                        1.0,
                        zb[:],
                        op0=ALU.mult,
                        op1=ALU.mult,
                    )

                # this group's projection contribution (grp 0 overlaps
                # grp 1's attention compute)
                for m in range(4):
                    nc.tensor.matmul(
                        pjt[m // 2][:, (m % 2) * E:(m % 2 + 1) * E],
                        oTg[:, m * 128:(m + 1) * 128],
                        w_sb["p"][:, grp * E:(grp + 1) * E],
                        start=(grp == 0 and m % 2 == 0),
                        stop=(grp == 1 and m % 2 == 1),
                        skip_group_check=True,
                    )

            # ---------- phase 3: bias + DMA out ----------
            for m in range(TQ // 128):
                ob = outp.tile([128, E], F32, tag="outsb")
                nc.vector.tensor_tensor(
                    ob[:], pjt[m // 2][:, (m % 2) * E:(m % 2 + 1) * E],
                    bp_sb[:], ALU.add,
                )
                eng = nc.sync if m % 2 == 0 else nc.scalar
                eng.dma_start(
                    out=out[m * 128:(m + 1) * 128, :], in_=ob[:]
                )

    return nc


def get_graph():
    global _GRAPH
    if _GRAPH is None:
        nc = _build_graph()
        nc.compile()
        _GRAPH = nc
    return _GRAPH


def make_in_maps(query, key_, value, Wq, bq, Wk, bk, Wv, bv, Wp, bp):
    query = np.asarray(query, np.float32)
    key_ = np.asarray(key_, np.float32)
    value = np.asarray(value, np.float32)
    Wq, Wk, Wv, Wp = (np.asarray(w, np.float32) for w in (Wq, Wk, Wv, Wp))
    bq, bk, bv, bp = (np.asarray(b_, np.float32) for b_ in (bq, bk, bv, bp))

    wq_b = np.ascontiguousarray(Wq).astype(BF)
    wk_b = np.ascontiguousarray(Wk).astype(BF)
    wv_b = np.ascontiguousarray(Wv).astype(BF)
    wp_b = np.ascontiguousarray(Wp).astype(BF)
    xt = {}
    for b in range(B):
        xt[("q", b)] = np.ascontiguousarray(query[:, b, :].T).astype(BF)
        xt[("k", b)] = np.ascontiguousarray(key_[:, b, :].T).astype(BF)
        xt[("v", b)] = np.ascontiguousarray(value[:, b, :].T).astype(BF)

    in_maps = []
    for c in range(NCORES):
        b = c // SP
        p = c % SP
        m = {
            "xqt": np.ascontiguousarray(xt[("q", b)][:, p * TQ:(p + 1) * TQ]),
            "xkt": xt[("k", b)],
            "xvt": xt[("v", b)],
            "wq": wq_b,
            "wk": wk_b,
            "wv": wv_b,
            "wp": wp_b,
            "bq": bq.reshape(1, E).copy(),
            "bk": bk.reshape(1, E).copy(),
            "bv": bv.reshape(1, E).copy(),
            "bp": bp.reshape(1, E).copy(),
        }
        in_maps.append(m)
    return in_maps


def assemble(results):
    out_full = np.empty((L, B, E), np.float32)
    for c in range(NCORES):
        b = c // SP
        p = c % SP
        out_full[p * TQ:(p + 1) * TQ, b, :] = results[c]["out"]
    return out_full


def run(inputs, trace=False, **kw):
    nc = get_graph()
    in_maps = make_in_maps(**inputs)
    res = run_bass_kernel_spmd(
        nc, in_maps, core_ids=list(range(NCORES)), trace=trace, **kw
    )
    return res


def kernel(**inputs):
    res = run(inputs, trace=False)
    return assemble(res.results)


# revision 15
# speedup vs baseline: 1.0093x; 1.0093x over previous
"""Distributed attention kernel for 8 TRN2 NeuronCores.

Problem: L=2048, B=2, E=256, H=8 heads, D=32 head-dim, fp32.

Sharding: DP2 over batch x sequence-parallel-4 over query positions.
Core c handles batch c//4, query rows [512*(c%4), 512*(c%4+1)), ALL 8
heads. k/v projections are redundantly computed per batch group (cheap)
and NO collective is needed: each core owns a disjoint output block.

Key structure (v2):
  - Softmax exp is split across TWO engines: ScalarE runs exact table
    exp (with fused 1/sqrt(D) scale); VectorE runs a Schraudolph
    bit-trick exp (i16 = x*A + B, reinterpreted as bf16 bits) for a
    subset of chunks, calibrated for zero softmax-weighted bias.
  - PV uses v as the STATIONARY operand: per (head, tk-chunk) one
    [128]-contraction matmul streaming 512 q columns into a [33, 512]
    psum block accumulated across all 16 chunks. The ones column of
    each [v|1] slot yields the softmax denominator Z as psum row 32.
  - Normalization: rz = 1/Z (DVE reciprocal straight from psum row),
    broadcast to 32 partitions with a K=1 fp32r ones-matmul into the
    same psum bank (partitions 64:96), then one scalar_tensor_tensor
    per head writes normalized O^T directly into the projection's
    lhsT layout. No DMA transposes anywhere.
  - Projection accumulates per 4-head group; biases are exact.
"""

import os
import sys

import numpy as np

for _p in ("/opt/trn_rl_repo",):
    if _p not in sys.path and os.path.isdir(_p):
        sys.path.insert(0, _p)

import ml_dtypes

import concourse.bass as bass
import concourse.bacc as bacc
import concourse.mybir as mybir
import concourse.tile as tile
from concourse.bass_utils import run_bass_kernel_spmd

dt = mybir.dt
F32 = dt.float32
F32R = dt.float32r
BF16 = dt.bfloat16
I16 = dt.int16
AF = mybir.ActivationFunctionType
ALU = mybir.AluOpType
BF = ml_dtypes.bfloat16

L, B, E, H, D = 2048, 2, 256, 8, 32
SCALE = float(D) ** -0.5
NCORES = 8
SP = 4            # sequence-parallel ways
TQ = L // SP      # 512 query rows per core
NTK = L // 128    # 16 tk chunks
VW = H * (D + 1)  # v_buf cols per tk chunk: 8x [v_h | 1] = 264

# Schraudolph exp constants (bf16-bits variant): i16 = round(s*A + B)
# viewed as bf16 equals exp(s*SCALE) with ~1.8% rms ripple and zero
# softmax-weighted mean (C calibrated numerically).
A_SCH = SCALE * (2.0 ** 23 / np.log(2.0)) / 65536.0
B_SCH = 127.0 * 128.0 - 483000.0 / 65536.0

# Of the 64 (head, chunk-pair) exp tiles, this many go to VectorE
# (Schraudolph); the rest run exact exp on ScalarE.
N_DVE = int(os.environ.get('N_DVE', 19))
DVE_CHUNKS = {(i * 64) // N_DVE for i in range(N_DVE)}

_GRAPH = None


def _build_graph():
    nc = bacc.Bacc(
        "TRN2",
        target_bir_lowering=False,
        debug=False,
        enable_asserts=False,
        num_devices=NCORES,
    )

    xqt = nc.declare_dram_parameter("xqt", [E, TQ], BF16, isOutput=False).ap()
    xkt = nc.declare_dram_parameter("xkt", [E, L], BF16, isOutput=False).ap()
    xvt = nc.declare_dram_parameter("xvt", [E, L], BF16, isOutput=False).ap()
    wq = nc.declare_dram_parameter("wq", [E, E], BF16, isOutput=False).ap()
    wk = nc.declare_dram_parameter("wk", [E, E], BF16, isOutput=False).ap()
    wv = nc.declare_dram_parameter("wv", [E, E], BF16, isOutput=False).ap()
    wp = nc.declare_dram_parameter("wp", [E, E], BF16, isOutput=False).ap()
    bq = nc.declare_dram_parameter("bq", [1, E], F32, isOutput=False).ap()
    bk = nc.declare_dram_parameter("bk", [1, E], F32, isOutput=False).ap()
    bv = nc.declare_dram_parameter("bv", [1, E], F32, isOutput=False).ap()
    bp = nc.declare_dram_parameter("bp", [1, E], F32, isOutput=False).ap()
    out = nc.declare_dram_parameter("out", [TQ, E], F32, isOutput=True).ap()

    with tile.TileContext(nc) as tc:
        with (
            tc.tile_pool(name="persist", bufs=1) as pp,
            tc.tile_pool(name="pt", bufs=4) as ptp,
            tc.tile_pool(name="oT", bufs=2) as oTp,
            tc.tile_pool(name="rz", bufs=4) as rzp,
            tc.tile_pool(name="zb", bufs=2) as zbp,
            tc.tile_pool(name="outsb", bufs=4) as outp,
            tc.tile_pool(name="st", bufs=2, space="PSUM") as stp,
            tc.tile_pool(name="pv", bufs=2, space="PSUM") as pvp,
            tc.tile_pool(name="pj", bufs=2, space="PSUM") as pjp,
        ):
            # ---------- phase 0: loads ----------
            # warm the Exp activation table before the first real exp
            warm = pp.tile([1, 16], F32)
            nc.vector.memset(warm[:], 0.0)
            nc.scalar.activation(warm[:], warm[:], AF.Exp)

            # weights: tile [128, 2E]; slice e covers W rows [128e, 128e+128)
            w_sb = {}

            def load_w(name, wsrc):
                t = pp.tile([128, 2 * E], BF16, name=f"w{name}", tag=f"w{name}")
                nc.scalar.dma_start(
                    out=t[:].rearrange("p (e n) -> p e n", e=2),
                    in_=wsrc.rearrange("(e p) n -> p e n", p=128),
                )
                w_sb[name] = t

            load_w("k", wk)
            load_w("q", wq)

            # biases: bq/bk as per-partition columns [128, 2] (hc chunks);
            # bv/bp replicated across partitions
            bq_sb = pp.tile([128, 2], F32)
            nc.gpsimd.dma_start(
                out=bq_sb[:], in_=bq.rearrange("a (c p) -> p (a c)", p=128)
            )
            bk_sb = pp.tile([128, 2], F32)
            nc.gpsimd.dma_start(
                out=bk_sb[:], in_=bk.rearrange("a (c p) -> p (a c)", p=128)
            )
            bv_sb = pp.tile([128, E], F32)
            nc.gpsimd.dma_start(out=bv_sb[:], in_=bv.to_broadcast((128, E)))
            bp_sb = pp.tile([128, E], F32)
            nc.gpsimd.dma_start(out=bp_sb[:], in_=bp.to_broadcast((128, E)))

            # x.T streams: xk on SyncE queue, xq on VectorE, xv on GpSimd
            xk_sb = [
                pp.tile([128, L], BF16, name=f"xkt{e}", tag=f"xkt{e}")
                for e in range(2)
            ]
            for n in range(2):
                for e in range(2):
                    nc.sync.dma_start(
                        out=xk_sb[e][:, n * 1024:(n + 1) * 1024],
                        in_=xkt[e * 128:(e + 1) * 128, n * 1024:(n + 1) * 1024],
                    )
            xq_sb = []
            for e in range(2):
                t = pp.tile([128, TQ], BF16, name=f"xqt{e}", tag=f"xqt{e}")
                nc.sync.dma_start(out=t[:], in_=xqt[e * 128:(e + 1) * 128, :])
                xq_sb.append(t)
            load_w("v", wv)
            load_w("p", wp)
            xv_sb = [
                pp.tile([128, L], BF16, name=f"xvt{e}", tag=f"xvt{e}")
                for e in range(2)
            ]
            for n in range(2):
                for e in range(2):
                    nc.gpsimd.dma_start(
                        out=xv_sb[e][:, n * 1024:(n + 1) * 1024],
                        in_=xvt[e * 128:(e + 1) * 128, n * 1024:(n + 1) * 1024],
                    )

            # v_buf: per tk chunk, 8x [v_h (32) | 1] slots; set only the
            # ones columns (strided memset, 128 elements)
            v_buf = pp.tile([128, NTK * VW], BF16)
            nc.gpsimd.memset(
                v_buf[:].rearrange("p (t h w) -> p t h w", t=NTK, h=H)[
                    :, :, :, D:D + 1
                ],
                1.0,
            )

            # PE warm-up: ~5us of dependency-free matmuls during the input
            # DMA phase so the HAM clock gate releases (1.2 -> 2.4 GHz)
            # before the first real matmul issues. Reuses the pj psum pool.
            wsrc = pp.tile([32, 128], BF16)
            nc.vector.memset(wsrc[:], 0.0)
            for w in range(48):
                wps = pjp.tile([128, 512], F32, tag="pj")
                nc.tensor.matmul(
                    wps[0:16, 0:128], wsrc[:, 0:16], wsrc[:, :],
                    start=True, stop=True,
                )

            # ---------- phase 1: projections ----------
            # kT: [256 head-dims, 2048] as four [64, 2048] tiles
            # (2 heads per tile at partition bases 0/32)
            ncopy = 0

            def copy_bias(dst, src, bias_ap):
                # psum->sbuf copy + per-partition bias, alternating engines
                nonlocal ncopy
                if os.environ.get('ALL_DVE_COPY') or ncopy % 2 == 0:
                    nc.vector.tensor_scalar_add(dst, src, bias_ap)
                else:
                    nc.scalar.activation(dst, src, AF.Identity, bias=bias_ap)
                ncopy += 1

            kT = [pp.tile([64, L], BF16, name=f"kT{pc}", tag=f"kT{pc}")
                  for pc in range(4)]
            for hc in range(2):
                for n in range(L // 512):
                    ps = pjp.tile([128, 512], F32, tag="pj")
                    for e in range(2):
                        nc.tensor.matmul(
                            ps[:],
                            w_sb["k"][:, e * E + hc * 128: e * E + (hc + 1) * 128],
                            xk_sb[e][:, n * 512:(n + 1) * 512],
                            start=(e == 0),
                            stop=(e == 1),
                        )
                    for half in range(2):
                        copy_bias(
                            kT[2 * hc + half][:, n * 512:(n + 1) * 512],
                            ps[half * 64:(half + 1) * 64, :],
                            bk_sb[half * 64:(half + 1) * 64, hc:hc + 1],
                        )

            # q.T slice: four [64, 512] tiles
            qT = [pp.tile([64, TQ], BF16, name=f"qT{pc}", tag=f"qT{pc}")
                  for pc in range(4)]
            for hc in range(2):
                ps = pjp.tile([128, TQ], F32, tag="pj")
                for e in range(2):
                    nc.tensor.matmul(
                        ps[:],
                        w_sb["q"][:, e * E + hc * 128: e * E + (hc + 1) * 128],
                        xq_sb[e][:, :],
                        start=(e == 0),
                        stop=(e == 1),
                    )
                for half in range(2):
                    copy_bias(
                        qT[2 * hc + half][:, :],
                        ps[half * 64:(half + 1) * 64, :],
                        bq_sb[half * 64:(half + 1) * 64, hc:hc + 1],
                    )

            # v (natural layout): per tk chunk -> strided scatter into the
            # [v|1] slots fused with the bias add (one DVE op per chunk)
            for t in range(NTK):
                ps = pjp.tile([128, E], F32, tag="pj")
                for e in range(2):
                    nc.tensor.matmul(
                        ps[:],
                        xv_sb[e][:, t * 128:(t + 1) * 128],
                        w_sb["v"][:, e * E:(e + 1) * E],
                        start=(e == 0),
                        stop=(e == 1),
                    )
                nc.vector.tensor_tensor(
                    v_buf[:, t * VW:(t + 1) * VW].rearrange(
                        "p (h w) -> p h w", h=H
                    )[:, :, 0:D],
                    ps[:].rearrange("p (h w) -> p h w", h=H),
                    bv_sb[:].rearrange("p (h w) -> p h w", h=H),
                    ALU.add,
                )

            # proj psum: two [128, 512] tiles hold the four [128, 256]
            # tq-chunk partials, accumulated across both head groups
            pjt = [pjp.tile([128, 2 * E], F32, name=f"pjt{i}", tag="pj")
                   for i in range(2)]

            # ---------- phase 2: attention ----------
            chunk_idx = 0
            for grp in range(2):
                oTg = oTp.tile([128, TQ], BF16, tag="oT")
                for u in range(4):
                    h = grp * 4 + u
                    hc, hr = h // 2, (h % 2) * D
                    pv = pvp.tile([D + 1, TQ], F32, tag="pv")
                    for g in range(NTK // 2):
                        st = stp.tile([128, 1024], F32, tag="st")
                        for i in range(2):
                            tk = 2 * g + i
                            nc.tensor.matmul(
                                st[:, i * 512:(i + 1) * 512],
                                kT[hc][hr:hr + D, tk * 128:(tk + 1) * 128],
                                qT[hc][hr:hr + D, :],
                                start=True,
                                stop=True,
                            )
                        pt = ptp.tile([128, 1024], BF16, tag="pt")
                        if chunk_idx in DVE_CHUNKS:
                            nc.vector.tensor_scalar(
                                pt[:].bitcast(I16), st[:], A_SCH, B_SCH,
                                op0=ALU.mult, op1=ALU.add,
                            )
                        else:
                            nc.scalar.activation(
                                pt[:], st[:], AF.Exp, scale=SCALE
                            )
                        chunk_idx += 1
                        for i in range(2):
                            tk = 2 * g + i
                            nc.tensor.matmul(
                                pv[0:D + 1, :],
                                v_buf[:, tk * VW + h * (D + 1):
                                      tk * VW + (h + 1) * (D + 1)],
                                pt[:, i * 512:(i + 1) * 512],
                                start=(g == 0 and i == 0),
                                stop=(g == NTK // 2 - 1 and i == 1),
                                skip_group_check=True,
                            )

                    # normalize head h: rz = 1/Z = exp(-ln Z) via two cheap
                    # ScalarE table ops (Ln and Exp share one table set, so
                    # no reloads), partition-broadcast to [32, TQ] on
                    # GpSimd, then one fused DVE multiply writes O^T into
                    # the projection layout (one PSUM operand per DVE op)
                    lnz = rzp.tile([1, TQ], F32, tag="lnz")
                    nc.scalar.activation(lnz[:], pv[D:D + 1, :], AF.Ln)
                    rz = rzp.tile([1, TQ], F32, tag="rz")
                    nc.scalar.activation(rz[:], lnz[:], AF.Exp, scale=-1.0)
                    zb = zbp.tile([D, TQ], F32, tag="zb")
                    nc.gpsimd.partition_broadcast(zb[:], rz[:])
                    nc.vector.scalar_tensor_tensor(
                        oTg[u * D:(u + 1) * D, :],
                        pv[0:D, :],
                        1.0,
                        zb[:],
                        op0=ALU.mult,
                        op1=ALU.mult,
                    )

                # this group's projection contribution (grp 0 overlaps
                # grp 1's attention compute)
                for m in range(4):
                    nc.tensor.matmul(
                        pjt[m // 2][:, (m % 2) * E:(m % 2 + 1) * E],
                        oTg[:, m * 128:(m + 1) * 128],
                        w_sb["p"][:, grp * E:(grp + 1) * E],
                        start=(grp == 0 and m % 2 == 0),
                        stop=(grp == 1 and m % 2 == 1),
                        skip_group_check=True,
                    )

            # ---------- phase 3: bias + DMA out ----------
            for m in range(TQ // 128):
                ob = outp.tile([128, E], F32, tag="outsb")
                nc.vector.tensor_tensor(
                    ob[:], pjt[m // 2][:, (m % 2) * E:(m % 2 + 1) * E],
                    bp_sb[:], ALU.add,
                )
                eng = nc.sync if m % 2 == 0 else nc.scalar
                eng.dma_start(
                    out=out[m * 128:(m + 1) * 128, :], in_=ob[:]
                )

    return nc


def get_graph():
    global _GRAPH
    if _GRAPH is None:
        nc = _build_graph()
        nc.compile()
        _GRAPH = nc
    return _GRAPH


def make_in_maps(query, key_, value, Wq, bq, Wk, bk, Wv, bv, Wp, bp):
    query = np.asarray(query, np.float32)
    key_ = np.asarray(key_, np.float32)
    value = np.asarray(value, np.float32)
    Wq, Wk, Wv, Wp = (np.asarray(w, np.float32) for w in (Wq, Wk, Wv, Wp))
    bq, bk, bv, bp = (np.asarray(b_, np.float32) for b_ in (bq, bk, bv, bp))

    wq_b = np.ascontiguousarray(Wq).astype(BF)
    wk_b = np.ascontiguousarray(Wk).astype(BF)
    wv_b = np.ascontiguousarray(Wv).astype(BF)
    wp_b = np.ascontiguousarray(Wp).astype(BF)
    xt = {}
    for b in range(B):
        xt[("q", b)] = np.ascontiguousarray(query[:, b, :].T).astype(BF)
        xt[("k", b)] = np.ascontiguousarray(key_[:, b, :].T).astype(BF)
        xt[("v", b)] = np.ascontiguousarray(value[:, b, :].T).astype(BF)

    in_maps = []
    for c in range(NCORES):
        b = c // SP
        p = c % SP
        m = {
            "xqt": np.ascontiguousarray(xt[("q", b)][:, p * TQ:(p + 1) * TQ]),
            "xkt": xt[("k", b)],
            "xvt": xt[("v", b)],
            "wq": wq_b,
            "wk": wk_b,
            "wv": wv_b,
            "wp": wp_b,
            "bq": bq.reshape(1, E).copy(),
            "bk": bk.reshape(1, E).copy(),
            "bv": bv.reshape(1, E).copy(),
            "bp": bp.reshape(1, E).copy(),
        }
        in_maps.append(m)
    return in_maps


def assemble(results):
    out_full = np.empty((L, B, E), np.float32)
    for c in range(NCORES):
        b = c // SP
        p = c % SP
        out_full[p * TQ:(p + 1) * TQ, b, :] = results[c]["out"]
    return out_full


def run(inputs, trace=False, **kw):
    nc = get_graph()
    in_maps = make_in_maps(**inputs)
    res = run_bass_kernel_spmd(
        nc, in_maps, core_ids=list(range(NCORES)), trace=trace, **kw
    )
    return res


def kernel(**inputs):
    res = run(inputs, trace=False)
    return assemble(res.results)


# revision 16
# speedup vs baseline: 1.0535x; 1.0438x over previous
"""Distributed attention kernel for 8 TRN2 NeuronCores.

Problem: L=2048, B=2, E=256, H=8 heads, D=32 head-dim, fp32.

Sharding: DP2 over batch x sequence-parallel-4 over query positions.
Core c handles batch c//4, query rows [512*(c%4), 512*(c%4+1)), ALL 8
heads. k/v projections are redundantly computed per batch group (cheap)
and NO collective is needed: each core owns a disjoint output block.

Key structure (v2):
  - Softmax exp is split across TWO engines: ScalarE runs exact table
    exp (with fused 1/sqrt(D) scale); VectorE runs a Schraudolph
    bit-trick exp (i16 = x*A + B, reinterpreted as bf16 bits) for a
    subset of chunks, calibrated for zero softmax-weighted bias.
  - PV uses v as the STATIONARY operand: per (head, tk-chunk) one
    [128]-contraction matmul streaming 512 q columns into a [33, 512]
    psum block accumulated across all 16 chunks. The ones column of
    each [v|1] slot yields the softmax denominator Z as psum row 32.
  - Normalization: rz = 1/Z (DVE reciprocal straight from psum row),
    broadcast to 32 partitions with a K=1 fp32r ones-matmul into the
    same psum bank (partitions 64:96), then one scalar_tensor_tensor
    per head writes normalized O^T directly into the projection's
    lhsT layout. No DMA transposes anywhere.
  - Projection accumulates per 4-head group; biases are exact.
"""

import os
import sys

import numpy as np

for _p in ("/opt/trn_rl_repo",):
    if _p not in sys.path and os.path.isdir(_p):
        sys.path.insert(0, _p)

import ml_dtypes

import concourse.bass as bass
import concourse.bacc as bacc
import concourse.mybir as mybir
import concourse.tile as tile
from concourse.bass_utils import run_bass_kernel_spmd

dt = mybir.dt
F32 = dt.float32
F32R = dt.float32r
BF16 = dt.bfloat16
I16 = dt.int16
AF = mybir.ActivationFunctionType
ALU = mybir.AluOpType
BF = ml_dtypes.bfloat16

L, B, E, H, D = 2048, 2, 256, 8, 32
SCALE = float(D) ** -0.5
NCORES = 8
SP = 4            # sequence-parallel ways
TQ = L // SP      # 512 query rows per core
NTK = L // 128    # 16 tk chunks
VW = H * (D + 1)  # v_buf cols per tk chunk: 8x [v_h | 1] = 264

# Schraudolph exp constants (bf16-bits variant): i16 = round(s*A + B)
# viewed as bf16 equals exp(s*SCALE) with ~1.8% rms ripple and zero
# softmax-weighted mean (C calibrated numerically).
A_SCH = SCALE * (2.0 ** 23 / np.log(2.0)) / 65536.0
B_SCH = 127.0 * 128.0 - 483000.0 / 65536.0

# Of the 64 (head, chunk-pair) exp tiles, this many go to VectorE
# (Schraudolph); the rest run exact exp on ScalarE.
N_DVE = int(os.environ.get('N_DVE', 19))
DVE_CHUNKS = {(i * 64) // N_DVE for i in range(N_DVE)}

_GRAPH = None


def _build_graph():
    nc = bacc.Bacc(
        "TRN2",
        target_bir_lowering=False,
        debug=False,
        enable_asserts=False,
        num_devices=NCORES,
    )

    xqt = nc.declare_dram_parameter("xqt", [E, TQ], BF16, isOutput=False).ap()
    xkt = nc.declare_dram_parameter("xkt", [E, L], BF16, isOutput=False).ap()
    xvt = nc.declare_dram_parameter("xvt", [E, L], BF16, isOutput=False).ap()
    wq = nc.declare_dram_parameter("wq", [E, E], BF16, isOutput=False).ap()
    wk = nc.declare_dram_parameter("wk", [E, E], BF16, isOutput=False).ap()
    wv = nc.declare_dram_parameter("wv", [E, E], BF16, isOutput=False).ap()
    wp = nc.declare_dram_parameter("wp", [E, E], BF16, isOutput=False).ap()
    bq = nc.declare_dram_parameter("bq", [1, E], F32, isOutput=False).ap()
    bk = nc.declare_dram_parameter("bk", [1, E], F32, isOutput=False).ap()
    bv = nc.declare_dram_parameter("bv", [1, E], F32, isOutput=False).ap()
    bp = nc.declare_dram_parameter("bp", [1, E], F32, isOutput=False).ap()
    out = nc.declare_dram_parameter("out", [TQ, E], F32, isOutput=True).ap()

    with tile.TileContext(nc) as tc:
        with (
            tc.tile_pool(name="persist", bufs=1) as pp,
            tc.tile_pool(name="pt", bufs=4) as ptp,
            tc.tile_pool(name="oT", bufs=2) as oTp,
            tc.tile_pool(name="rz", bufs=4) as rzp,
            tc.tile_pool(name="zb", bufs=2) as zbp,
            tc.tile_pool(name="outsb", bufs=4) as outp,
            tc.tile_pool(name="st", bufs=2, space="PSUM") as stp,
            tc.tile_pool(name="pv", bufs=2, space="PSUM") as pvp,
            tc.tile_pool(name="pj", bufs=2, space="PSUM") as pjp,
        ):
            # ---------- phase 0: loads ----------
            # warm the Exp activation table before the first real exp
            warm = pp.tile([1, 16], F32)
            nc.vector.memset(warm[:], 0.0)
            nc.scalar.activation(warm[:], warm[:], AF.Exp)

            # weights: tile [128, 2E]; slice e covers W rows [128e, 128e+128)
            w_sb = {}

            def load_w(name, wsrc):
                t = pp.tile([128, 2 * E], BF16, name=f"w{name}", tag=f"w{name}")
                nc.scalar.dma_start(
                    out=t[:].rearrange("p (e n) -> p e n", e=2),
                    in_=wsrc.rearrange("(e p) n -> p e n", p=128),
                )
                w_sb[name] = t

            load_w("k", wk)
            load_w("q", wq)

            # biases: bq/bk as per-partition columns [128, 2] (hc chunks);
            # bv/bp replicated across partitions
            bq_sb = pp.tile([128, 2], F32)
            nc.gpsimd.dma_start(
                out=bq_sb[:], in_=bq.rearrange("a (c p) -> p (a c)", p=128)
            )
            bk_sb = pp.tile([128, 2], F32)
            nc.gpsimd.dma_start(
                out=bk_sb[:], in_=bk.rearrange("a (c p) -> p (a c)", p=128)
            )
            bv_sb = pp.tile([128, E], F32)
            nc.gpsimd.dma_start(out=bv_sb[:], in_=bv.to_broadcast((128, E)))
            bp_sb = pp.tile([128, E], F32)
            nc.gpsimd.dma_start(out=bp_sb[:], in_=bp.to_broadcast((128, E)))

            # x.T streams: xk on SyncE queue, xq on VectorE, xv on GpSimd
            xk_sb = [
                pp.tile([128, L], BF16, name=f"xkt{e}", tag=f"xkt{e}")
                for e in range(2)
            ]
            for n in range(2):
                for e in range(2):
                    nc.sync.dma_start(
                        out=xk_sb[e][:, n * 1024:(n + 1) * 1024],
                        in_=xkt[e * 128:(e + 1) * 128, n * 1024:(n + 1) * 1024],
                    )
            xq_sb = []
            for e in range(2):
                t = pp.tile([128, TQ], BF16, name=f"xqt{e}", tag=f"xqt{e}")
                nc.sync.dma_start(out=t[:], in_=xqt[e * 128:(e + 1) * 128, :])
                xq_sb.append(t)
            load_w("v", wv)
            load_w("p", wp)
            xv_sb = [
                pp.tile([128, L], BF16, name=f"xvt{e}", tag=f"xvt{e}")
                for e in range(2)
            ]
            for n in range(2):
                for e in range(2):
                    nc.gpsimd.dma_start(
                        out=xv_sb[e][:, n * 1024:(n + 1) * 1024],
                        in_=xvt[e * 128:(e + 1) * 128, n * 1024:(n + 1) * 1024],
                    )

            # v_buf: per tk chunk, 8x [v_h (32) | 1] slots; set only the
            # ones columns (strided memset, 128 elements)
            v_buf = pp.tile([128, NTK * VW], BF16)
            nc.gpsimd.memset(
                v_buf[:].rearrange("p (t h w) -> p t h w", t=NTK, h=H)[
                    :, :, :, D:D + 1
                ],
                1.0,
            )

            # PE warm-up: ~5us of dependency-free matmuls during the input
            # DMA phase so the HAM clock gate releases (1.2 -> 2.4 GHz)
            # before the first real matmul issues. Reuses the pj psum pool.
            wsrc = pp.tile([32, 128], BF16)
            nc.vector.memset(wsrc[:], 0.0)
            for w in range(48):
                wps = pjp.tile([128, 512], F32, tag="pj")
                nc.tensor.matmul(
                    wps[0:16, 0:128], wsrc[:, 0:16], wsrc[:, :],
                    start=True, stop=True,
                )

            # ---------- phase 1: projections ----------
            # kT: [256 head-dims, 2048] as four [64, 2048] tiles
            # (2 heads per tile at partition bases 0/32)
            ncopy = 0

            def copy_bias(dst, src, bias_ap):
                # psum->sbuf copy + per-partition bias, alternating engines
                nonlocal ncopy
                if os.environ.get('ALL_DVE_COPY') or ncopy % 2 == 0:
                    nc.vector.tensor_scalar_add(dst, src, bias_ap)
                else:
                    nc.scalar.activation(dst, src, AF.Identity, bias=bias_ap)
                ncopy += 1

            kT = [pp.tile([64, L], BF16, name=f"kT{pc}", tag=f"kT{pc}")
                  for pc in range(4)]
            for hc in range(2):
                for n in range(L // 512):
                    ps = pjp.tile([128, 512], F32, tag="pj")
                    for e in range(2):
                        nc.tensor.matmul(
                            ps[:],
                            w_sb["k"][:, e * E + hc * 128: e * E + (hc + 1) * 128],
                            xk_sb[e][:, n * 512:(n + 1) * 512],
                            start=(e == 0),
                            stop=(e == 1),
                        )
                    for half in range(2):
                        copy_bias(
                            kT[2 * hc + half][:, n * 512:(n + 1) * 512],
                            ps[half * 64:(half + 1) * 64, :],
                            bk_sb[half * 64:(half + 1) * 64, hc:hc + 1],
                        )

            # q.T slice: four [64, 512] tiles
            qT = [pp.tile([64, TQ], BF16, name=f"qT{pc}", tag=f"qT{pc}")
                  for pc in range(4)]
            for hc in range(2):
                ps = pjp.tile([128, TQ], F32, tag="pj")
                for e in range(2):
                    nc.tensor.matmul(
                        ps[:],
                        w_sb["q"][:, e * E + hc * 128: e * E + (hc + 1) * 128],
                        xq_sb[e][:, :],
                        start=(e == 0),
                        stop=(e == 1),
                    )
                for half in range(2):
                    copy_bias(
                        qT[2 * hc + half][:, :],
                        ps[half * 64:(half + 1) * 64, :],
                        bq_sb[half * 64:(half + 1) * 64, hc:hc + 1],
                    )

            # v (natural layout): per tk chunk -> strided scatter into the
            # [v|1] slots fused with the bias add (one DVE op per chunk)
            for t in range(NTK):
                ps = pjp.tile([128, E], F32, tag="pj")
                for e in range(2):
                    nc.tensor.matmul(
                        ps[:],
                        xv_sb[e][:, t * 128:(t + 1) * 128],
                        w_sb["v"][:, e * E:(e + 1) * E],
                        start=(e == 0),
                        stop=(e == 1),
                    )
                nc.vector.tensor_tensor(
                    v_buf[:, t * VW:(t + 1) * VW].rearrange(
                        "p (h w) -> p h w", h=H
                    )[:, :, 0:D],
                    ps[:].rearrange("p (h w) -> p h w", h=H),
                    bv_sb[:].rearrange("p (h w) -> p h w", h=H),
                    ALU.add,
                )

            # proj psum: two [128, 512] tiles hold the four [128, 256]
            # tq-chunk partials, accumulated across both head groups
            pjt = [pjp.tile([128, 2 * E], F32, name=f"pjt{i}", tag="pj")
                   for i in range(2)]

            # ---------- phase 2: attention ----------
            # Heads are processed in PAIRS, chunk-lockstep: per tk-chunk
            # pair g the PE issues S(A) S(A) S(B) S(B) PV(A) PV(A) PV(B)
            # PV(B) while head A's exp runs on ScalarE and head B's runs
            # CONCURRENTLY on VectorE (Schraudolph). The PE thus always
            # has an independent matmul stream -> no dependency bubbles
            # -> the HAM clock gate stays released (2.4 GHz).
            # Both heads' [O|Z] blocks share one psum bank: A at rows
            # 0:33, B at rows 64:97.
            for grp in range(2):
                oTg = oTp.tile([128, TQ], BF16, tag="oT")
                for up in range(2):  # head pair within group
                    hA = grp * 4 + 2 * up      # exp on ScalarE
                    hB = hA + 1                # exp on VectorE
                    pv = pvp.tile([97, TQ], F32, tag="pv")
                    pvo = {hA: 0, hB: 64}
                    sts = {}
                    for g in range(NTK // 2):
                        for h in (hA, hB):
                            hc, hr = h // 2, (h % 2) * D
                            st = stp.tile([128, 1024], F32, tag="st")
                            sts[h] = st
                            for i in range(2):
                                tk = 2 * g + i
                                nc.tensor.matmul(
                                    st[:, i * 512:(i + 1) * 512],
                                    kT[hc][hr:hr + D, tk * 128:(tk + 1) * 128],
                                    qT[hc][hr:hr + D, :],
                                    start=True,
                                    stop=True,
                                )
                        ptA = ptp.tile([128, 1024], BF16, tag="pt")
                        nc.scalar.activation(
                            ptA[:], sts[hA][:], AF.Exp, scale=SCALE
                        )
                        ptB = ptp.tile([128, 1024], BF16, tag="pt")
                        nc.vector.tensor_scalar(
                            ptB[:].bitcast(I16), sts[hB][:], A_SCH, B_SCH,
                            op0=ALU.mult, op1=ALU.add,
                        )
                        for h, pt in ((hA, ptA), (hB, ptB)):
                            o = pvo[h]
                            for i in range(2):
                                tk = 2 * g + i
                                nc.tensor.matmul(
                                    pv[o:o + D + 1, :],
                                    v_buf[:, tk * VW + h * (D + 1):
                                          tk * VW + (h + 1) * (D + 1)],
                                    pt[:, i * 512:(i + 1) * 512],
                                    start=(g == 0 and i == 0),
                                    stop=(g == NTK // 2 - 1 and i == 1),
                                    skip_group_check=True,
                                )

                    # normalize both heads: rz = 1/Z = exp(-ln Z) via two
                    # ScalarE table ops (Ln and Exp share one table set),
                    # partition-broadcast on GpSimd, then one fused DVE
                    # multiply per head writes O^T into the projection
                    # layout (one PSUM operand per DVE op)
                    for h in (hA, hB):
                        o = pvo[h]
                        u = h - grp * 4
                        lnz = rzp.tile([1, TQ], F32, tag="lnz")
                        nc.scalar.activation(
                            lnz[:], pv[o + D:o + D + 1, :], AF.Ln
                        )
                        rz = rzp.tile([1, TQ], F32, tag="rz")
                        nc.scalar.activation(rz[:], lnz[:], AF.Exp, scale=-1.0)
                        zb = zbp.tile([D, TQ], F32, tag="zb")
                        nc.gpsimd.partition_broadcast(zb[:], rz[:])
                        nc.vector.scalar_tensor_tensor(
                            oTg[u * D:(u + 1) * D, :],
                            pv[o:o + D, :],
                            1.0,
                            zb[:],
                            op0=ALU.mult,
                            op1=ALU.mult,
                        )

                # this group's projection contribution (grp 0 overlaps
                # grp 1's attention compute)
                for m in range(4):
                    nc.tensor.matmul(
                        pjt[m // 2][:, (m % 2) * E:(m % 2 + 1) * E],
                        oTg[:, m * 128:(m + 1) * 128],
                        w_sb["p"][:, grp * E:(grp + 1) * E],
                        start=(grp == 0 and m % 2 == 0),
                        stop=(grp == 1 and m % 2 == 1),
                        skip_group_check=True,
                    )

            # ---------- phase 3: bias + DMA out ----------
            for m in range(TQ // 128):
                ob = outp.tile([128, E], F32, tag="outsb")
                nc.vector.tensor_tensor(
                    ob[:], pjt[m // 2][:, (m % 2) * E:(m % 2 + 1) * E],
                    bp_sb[:], ALU.add,
                )
                eng = nc.sync if m % 2 == 0 else nc.scalar
                eng.dma_start(
                    out=out[m * 128:(m + 1) * 128, :], in_=ob[:]
                )

    return nc


def get_graph():
    global _GRAPH
    if _GRAPH is None:
        nc = _build_graph()
        nc.compile()
        _GRAPH = nc
    return _GRAPH


def make_in_maps(query, key_, value, Wq, bq, Wk, bk, Wv, bv, Wp, bp):
    query = np.asarray(query, np.float32)
    key_ = np.asarray(key_, np.float32)
    value = np.asarray(value, np.float32)
    Wq, Wk, Wv, Wp = (np.asarray(w, np.float32) for w in (Wq, Wk, Wv, Wp))
    bq, bk, bv, bp = (np.asarray(b_, np.float32) for b_ in (bq, bk, bv, bp))

    wq_b = np.ascontiguousarray(Wq).astype(BF)
    wk_b = np.ascontiguousarray(Wk).astype(BF)
    wv_b = np.ascontiguousarray(Wv).astype(BF)
    wp_b = np.ascontiguousarray(Wp).astype(BF)
    xt = {}
    for b in range(B):
        xt[("q", b)] = np.ascontiguousarray(query[:, b, :].T).astype(BF)
        xt[("k", b)] = np.ascontiguousarray(key_[:, b, :].T).astype(BF)
        xt[("v", b)] = np.ascontiguousarray(value[:, b, :].T).astype(BF)

    in_maps = []
    for c in range(NCORES):
        b = c // SP
        p = c % SP
        m = {
            "xqt": np.ascontiguousarray(xt[("q", b)][:, p * TQ:(p + 1) * TQ]),
            "xkt": xt[("k", b)],
            "xvt": xt[("v", b)],
            "wq": wq_b,
            "wk": wk_b,
            "wv": wv_b,
            "wp": wp_b,
            "bq": bq.reshape(1, E).copy(),
            "bk": bk.reshape(1, E).copy(),
            "bv": bv.reshape(1, E).copy(),
            "bp": bp.reshape(1, E).copy(),
        }
        in_maps.append(m)
    return in_maps


def assemble(results):
    out_full = np.empty((L, B, E), np.float32)
    for c in range(NCORES):
        b = c // SP
        p = c % SP
        out_full[p * TQ:(p + 1) * TQ, b, :] = results[c]["out"]
    return out_full


def run(inputs, trace=False, **kw):
    nc = get_graph()
    in_maps = make_in_maps(**inputs)
    res = run_bass_kernel_spmd(
        nc, in_maps, core_ids=list(range(NCORES)), trace=trace, **kw
    )
    return res


def kernel(**inputs):
    res = run(inputs, trace=False)
    return assemble(res.results)


# revision 24
# speedup vs baseline: 1.1432x; 1.0851x over previous
"""Distributed attention kernel for 8 TRN2 NeuronCores.

Problem: L=2048, B=2, E=256, H=8 heads, D=32 head-dim, fp32.

Sharding: DP2 over batch x sequence-parallel-4 over query positions.
Core c handles batch c//4, query rows [512*(c%4), 512*(c%4+1)), ALL 8
heads. k/v projections are redundantly computed per batch group (cheap)
and NO collective is needed: each core owns a disjoint output block.

v3 — designed around the PE HAM clock gate: the Tensor engine only
un-throttles (1.2 -> 2.4 GHz) when matmuls keep >64 contraction rows
active, so EVERY hot matmul here uses K=128:

  - S: BLOCK-DIAGONAL stationary. kT is laid out as [128, 128] tiles
    whose four diagonal [32d x 32tk] blocks are four heads' keys for
    one 32-wide tk chunk. One matmul against the 4-head stacked qT
    [128, 512] computes S^T for 4 heads x 32 tk x 512 q at K=128 --
    the same total moving columns as per-head K=32 matmuls, but 4x
    the array activity and 2x the sustained clock.
  - exp: split between ScalarE (exact table exp, fused scale) and
    VectorE (Schraudolph bit-trick into bf16 bits, zero-bias
    calibrated), round-robin over chunks.
  - PV: P-stationary block-diagonal. The stationary is a [128, 128]
    pt slice (K=128); the moving operand v_mov packs four diagonal
    [32tk x 33] blocks [V_h | 1] so one pass yields O in natural
    orientation for 4 heads PLUS the softmax denominators Z.
  - normalize: per-partition (q) reciprocal [128, 4] + tiny
    per-head scaled copies -- all cheap, then xbar DMA transposes
    produce O^T for the projection.
"""

import os
import sys

import numpy as np

for _p in ("/opt/trn_rl_repo",):
    if _p not in sys.path and os.path.isdir(_p):
        sys.path.insert(0, _p)

import ml_dtypes

import concourse.bass as bass
import concourse.bacc as bacc
import concourse.mybir as mybir
import concourse.tile as tile
from concourse.bass_utils import run_bass_kernel_spmd

dt = mybir.dt
F32 = dt.float32
BF16 = dt.bfloat16
I16 = dt.int16
AF = mybir.ActivationFunctionType
ALU = mybir.AluOpType
BF = ml_dtypes.bfloat16

L, B, E, H, D = 2048, 2, 256, 8, 32
SCALE = float(D) ** -0.5
NCORES = 8
SP = 4            # sequence-parallel ways
TQ = L // SP      # 512 query rows per core
NC32 = L // 32    # 64 tk chunks of 32
VW = H * (D + 1)  # v_buf cols per tk chunk of 128: 8x [v_h | 1] = 264
BW = 4 * (D + 1)  # v_mov cols per tk32 chunk: 4x [v_h | 1] = 132

# Schraudolph exp constants (bf16-bits variant): i16 = round(s*A + B)
# viewed as bf16 equals exp(s*SCALE), ~1.8% rms ripple, zero
# softmax-weighted mean (C calibrated numerically).
A_SCH = SCALE * (2.0 ** 23 / np.log(2.0)) / 65536.0
B_SCH = 127.0 * 128.0 - 483000.0 / 65536.0

# Of the 64 (group, chunk-pair) exp tiles, this many go to VectorE
# (Schraudolph); the rest run exact exp on ScalarE.
N_DVE = int(os.environ.get("N_DVE", 26))
DVE_CHUNKS = {(i * 64) // N_DVE for i in range(N_DVE)}

_GRAPH = None


def _build_graph():
    nc = bacc.Bacc(
        "TRN2",
        target_bir_lowering=False,
        debug=False,
        enable_asserts=False,
        num_devices=NCORES,
    )

    xqt = nc.declare_dram_parameter("xqt", [E, TQ], BF16, isOutput=False).ap()
    xkt = nc.declare_dram_parameter("xkt", [E, L], BF16, isOutput=False).ap()
    xvt = nc.declare_dram_parameter("xvt", [E, L], BF16, isOutput=False).ap()
    wq = nc.declare_dram_parameter("wq", [E, E], BF16, isOutput=False).ap()
    wk = nc.declare_dram_parameter("wk", [E, E], BF16, isOutput=False).ap()
    wv = nc.declare_dram_parameter("wv", [E, E], BF16, isOutput=False).ap()
    wp = nc.declare_dram_parameter("wp", [E, E], BF16, isOutput=False).ap()
    bq = nc.declare_dram_parameter("bq", [1, E], F32, isOutput=False).ap()
    bk = nc.declare_dram_parameter("bk", [1, E], F32, isOutput=False).ap()
    bv = nc.declare_dram_parameter("bv", [1, E], F32, isOutput=False).ap()
    bp = nc.declare_dram_parameter("bp", [1, E], F32, isOutput=False).ap()
    out = nc.declare_dram_parameter("out", [TQ, E], F32, isOutput=True).ap()

    with tile.TileContext(nc) as tc:
        with (
            tc.tile_pool(name="persist", bufs=1) as pp,
            tc.tile_pool(name="pt", bufs=4) as ptp,
            tc.tile_pool(name="oT", bufs=2) as oTp,
            tc.tile_pool(name="onat", bufs=2) as onp,
            tc.tile_pool(name="rz", bufs=4) as rzp,
            tc.tile_pool(name="outsb", bufs=4) as outp,
            tc.tile_pool(name="st", bufs=2, space="PSUM") as stp,
            tc.tile_pool(name="pv", bufs=2, space="PSUM") as pvp,
            tc.tile_pool(name="pj", bufs=2, space="PSUM") as pjp,
        ):
            # ---------- phase 0: loads ----------
            # warm the Exp activation table before the first real exp
            warm = pp.tile([1, 16], F32)
            nc.vector.memset(warm[:], 0.0)
            nc.scalar.activation(warm[:], warm[:], AF.Exp)

            # weights: tile [128, 2E]; slice e covers W rows [128e, 128e+128)
            w_sb = {}

            def load_w(name, wsrc):
                t = pp.tile([128, 2 * E], BF16, name=f"w{name}", tag=f"w{name}")
                nc.scalar.dma_start(
                    out=t[:].rearrange("p (e n) -> p e n", e=2),
                    in_=wsrc.rearrange("(e p) n -> p e n", p=128),
                )
                w_sb[name] = t

            load_w("k", wk)
            load_w("q", wq)

            # biases: bq/bk as per-partition columns [128, 2] (hc chunks);
            # bv/bp replicated across partitions
            bq_sb = pp.tile([128, 2], F32)
            nc.gpsimd.dma_start(
                out=bq_sb[:], in_=bq.rearrange("a (c p) -> p (a c)", p=128)
            )
            bk_sb = pp.tile([128, 2], F32)
            nc.gpsimd.dma_start(
                out=bk_sb[:], in_=bk.rearrange("a (c p) -> p (a c)", p=128)
            )
            bv_sb = pp.tile([128, E], F32)
            nc.gpsimd.dma_start(out=bv_sb[:], in_=bv.to_broadcast((128, E)))
            bp_sb = pp.tile([128, E], F32)
            nc.gpsimd.dma_start(out=bp_sb[:], in_=bp.to_broadcast((128, E)))

            # x.T streams: xk + xq on SyncE queue, xv on GpSimd
            xk_sb = [
                pp.tile([128, L], BF16, name=f"xkt{e}", tag=f"xkt{e}")
                for e in range(2)
            ]
            for n in range(2):
                for e in range(2):
                    nc.sync.dma_start(
                        out=xk_sb[e][:, n * 1024:(n + 1) * 1024],
                        in_=xkt[e * 128:(e + 1) * 128, n * 1024:(n + 1) * 1024],
                    )
            xq_sb = []
            for e in range(2):
                t = pp.tile([128, TQ], BF16, name=f"xqt{e}", tag=f"xqt{e}")
                nc.sync.dma_start(out=t[:], in_=xqt[e * 128:(e + 1) * 128, :])
                xq_sb.append(t)
            load_w("v", wv)
            load_w("p", wp)
            xv_sb = [
                pp.tile([128, L], BF16, name=f"xvt{e}", tag=f"xvt{e}")
                for e in range(2)
            ]
            for n in range(2):
                for e in range(2):
                    nc.gpsimd.dma_start(
                        out=xv_sb[e][:, n * 1024:(n + 1) * 1024],
                        in_=xvt[e * 128:(e + 1) * 128, n * 1024:(n + 1) * 1024],
                    )

            # v_buf (staging, natural layout): per tk-128 chunk, 8x
            # [v_h (32) | 1] slots; only the ones columns are memset
            v_buf = pp.tile([128, (L // 128) * VW], BF16)
            nc.gpsimd.memset(
                v_buf[:].rearrange("p (t h w) -> p t h w", t=L // 128, h=H)[
                    :, :, :, D:D + 1
                ],
                1.0,
            )

            # block-diagonal operand homes; zero-filled on GpSimd first
            # (kbd[0] earliest: S for group 0 starts as soon as it and the
            # kT copies land)
            kbd = [pp.tile([128, NC32 * 128], BF16, name=f"kbd{g}",
                           tag=f"kbd{g}") for g in range(2)]
            vm = [pp.tile([128, NC32 * BW], BF16, name=f"vm{g}",
                          tag=f"vm{g}") for g in range(2)]
            nc.gpsimd.memset(kbd[0][:], 0.0)
            nc.gpsimd.memset(vm[0][:], 0.0)
            nc.gpsimd.memset(kbd[1][:], 0.0)
            nc.gpsimd.memset(vm[1][:], 0.0)

            # PE warm-up: dependency-free K=128 matmuls during the DMA
            # phase so the HAM clock gate releases before real work
            wsrc = pp.tile([128, 512], BF16)
            nc.vector.memset(wsrc[:], 0.0)
            for w in range(20):
                wps = pjp.tile([128, 512], F32, tag="pj")
                nc.tensor.matmul(
                    wps[:], wsrc[:, 0:128], wsrc[:, :], start=True, stop=True,
                )

            # ---------- phase 1: projections ----------
            ncopy = 0

            def copy_bias(dst, src, bias_ap):
                # psum->sbuf copy + per-partition bias, alternating engines
                nonlocal ncopy
                if ncopy % 2 == 0:
                    nc.vector.tensor_scalar_add(dst, src, bias_ap)
                else:
                    nc.scalar.activation(dst, src, AF.Identity, bias=bias_ap)
                ncopy += 1

            # kT: four [64, 2048] tiles (2 heads per tile at bases 0/32)
            kT = [pp.tile([64, L], BF16, name=f"kT{pc}", tag=f"kT{pc}")
                  for pc in range(4)]
            for hc in range(2):
                for n in range(L // 512):
                    ps = pjp.tile([128, 512], F32, tag="pj")
                    for e in range(2):
                        nc.tensor.matmul(
                            ps[:],
                            w_sb["k"][:, e * E + hc * 128: e * E + (hc + 1) * 128],
                            xk_sb[e][:, n * 512:(n + 1) * 512],
                            start=(e == 0),
                            stop=(e == 1),
                        )
                    for half in range(2):
                        copy_bias(
                            kT[2 * hc + half][:, n * 512:(n + 1) * 512],
                            ps[half * 64:(half + 1) * 64, :],
                            bk_sb[half * 64:(half + 1) * 64, hc:hc + 1],
                        )

            # build kbd[g]: per head, one strided SBUF->SBUF DMA scatters
            # kT's [32, (c, t')] rows into the diagonal block position
            for g in range(2):
                for hl in range(4):
                    h = g * 4 + hl
                    pc, hr = h // 2, (h % 2) * D
                    eng = ((nc.sync, nc.scalar)[hl % 2] if g == 0
                           else nc.gpsimd)
                    eng.dma_start(
                        out=kbd[g][hl * D:(hl + 1) * D, :].rearrange(
                            "p (c i t) -> p c i t", c=NC32, i=4
                        )[:, :, hl, :],
                        in_=kT[pc][hr:hr + D, :].rearrange(
                            "p (c t) -> p c t", c=NC32
                        ),
                    )

            # qT groups: [128, 512] stacked 4 heads (= one psum block)
            qTg = []
            for g in range(2):
                t = pp.tile([128, TQ], BF16, name=f"qTg{g}", tag=f"qTg{g}")
                ps = pjp.tile([128, TQ], F32, tag="pj")
                for e in range(2):
                    nc.tensor.matmul(
                        ps[:],
                        w_sb["q"][:, e * E + g * 128: e * E + (g + 1) * 128],
                        xq_sb[e][:, :],
                        start=(e == 0),
                        stop=(e == 1),
                    )
                copy_bias(t[:], ps[:], bq_sb[:, g:g + 1])
                qTg.append(t)

            # v (natural layout): per tk-128 chunk -> strided scatter into
            # the [v|1] slots fused with the bias add (one DVE op each)
            for t in range(L // 128):
                ps = pjp.tile([128, E], F32, tag="pj")
                for e in range(2):
                    nc.tensor.matmul(
                        ps[:],
                        xv_sb[e][:, t * 128:(t + 1) * 128],
                        w_sb["v"][:, e * E:(e + 1) * E],
                        start=(e == 0),
                        stop=(e == 1),
                    )
                nc.vector.tensor_tensor(
                    v_buf[:, t * VW:(t + 1) * VW].rearrange(
                        "p (h w) -> p h w", h=H
                    )[:, :, 0:D],
                    ps[:].rearrange("p (h w) -> p h w", h=H),
                    bv_sb[:].rearrange("p (h w) -> p h w", h=H),
                    ALU.add,
                )

            # build vm[g]: per (head, C-half) one strided SBUF->SBUF DMA
            # moves [v_h|1] (incl. the ones column -> Z) into the diagonal
            # moving-block position
            for g in range(2):
                for hl in range(4):
                    h = g * 4 + hl
                    for q in range(4):
                        eng = ((nc.sync, nc.scalar)[(hl + q) % 2]
                               if g == 0 else nc.gpsimd)
                        eng.dma_start(
                            out=vm[g][hl * D:(hl + 1) * D, :]
                            .rearrange("p (C q2 i w) -> p C q2 i w",
                                       C=16, q2=4, i=4)[:, :, q, hl, :],
                            in_=v_buf[q * 32:(q + 1) * 32, :]
                            .rearrange("p (C hh w) -> p C hh w",
                                       C=16, hh=H)[:, :, h, :],
                        )

            # proj psum: two [128, 512] tiles hold the four [128, 256]
            # tq-chunk partials, accumulated across both head groups
            pjt = [pjp.tile([128, 2 * E], F32, name=f"pjt{i}", tag="pj")
                   for i in range(2)]

            # ---------- phase 2: attention ----------
            chunk_idx = 0
            for g in range(2):
                oTg = oTp.tile([128, TQ], BF16, tag="oT")
                onat = onp.tile([128, TQ], BF16, tag="onat")
                # 4 [128, 132] O|Z accumulators packed two per psum bank
                pvt = [pvp.tile([128, 2 * BW], F32, name=f"pvt{g}_{ii}",
                                tag="pv") for ii in range(2)]
                for j in range(NC32 // 2):
                    st = stp.tile([128, 1024], F32, tag="st")
                    for i in range(2):
                        c = 2 * j + i
                        nc.tensor.matmul(
                            st[:, i * 512:(i + 1) * 512],
                            kbd[g][:, c * 128:(c + 1) * 128],
                            qTg[g][:, :],
                            start=True,
                            stop=True,
                        )
                    pt = ptp.tile([128, 1024], BF16, tag="pt")
                    if chunk_idx in DVE_CHUNKS:
                        nc.vector.tensor_scalar(
                            pt[:].bitcast(I16), st[:], A_SCH, B_SCH,
                            op0=ALU.mult, op1=ALU.add,
                        )
                    else:
                        nc.scalar.activation(pt[:], st[:], AF.Exp, scale=SCALE)
                    chunk_idx += 1
                    for i in range(2):
                        c = 2 * j + i
                        for q in range(4):
                            nc.tensor.matmul(
                                pvt[q // 2][:, (q % 2) * BW:(q % 2 + 1) * BW],
                                pt[:, i * 512 + q * 128: i * 512 + (q + 1) * 128],
                                vm[g][:, c * BW:(c + 1) * BW],
                                start=(j == 0 and i == 0 and q % 2 == 0),
                                stop=(j == NC32 // 2 - 1 and i == 1
                                      and q % 2 == 1),
                                skip_group_check=True,
                            )

                # normalize: per q-chunk a [128, 4] strided reciprocal of
                # the Z columns, then 4 per-head scaled copies into onat
                for q in range(4):
                    po = pvt[q // 2][:, (q % 2) * BW:(q % 2 + 1) * BW]
                    rz = rzp.tile([128, 4], F32, tag="rz")
                    nc.vector.reciprocal(
                        rz[:], po.rearrange("p (u w) -> p u w", u=4)[:, :, D]
                    )
                    for u in range(4):
                        if (q * 4 + u) % 2 == 0:
                            nc.vector.tensor_scalar_mul(
                                onat[:, q * 128 + u * D: q * 128 + (u + 1) * D],
                                po[:, u * (D + 1): u * (D + 1) + D],
                                rz[:, u:u + 1],
                            )
                        else:
                            nc.scalar.activation(
                                onat[:, q * 128 + u * D: q * 128 + (u + 1) * D],
                                po[:, u * (D + 1): u * (D + 1) + D],
                                AF.Identity, scale=rz[:, u:u + 1],
                            )

                    # transpose [tq 128, 4x32 dv] -> [4x32 dv, tq 128]
                    eng = nc.sync if q % 2 == 0 else nc.scalar
                    eng.dma_start_transpose(
                        oTg[:, q * 128:(q + 1) * 128],
                        onat[:, q * 128:(q + 1) * 128],
                    )

                # this group's projection contribution
                for m in range(4):
                    nc.tensor.matmul(
                        pjt[m // 2][:, (m % 2) * E:(m % 2 + 1) * E],
                        oTg[:, m * 128:(m + 1) * 128],
                        w_sb["p"][:, g * E:(g + 1) * E],
                        start=(g == 0 and m % 2 == 0),
                        stop=(g == 1 and m % 2 == 1),
                        skip_group_check=True,
                    )

            # ---------- phase 3: bias + DMA out ----------
            for m in range(TQ // 128):
                ob = outp.tile([128, E], F32, tag="outsb")
                nc.vector.tensor_tensor(
                    ob[:], pjt[m // 2][:, (m % 2) * E:(m % 2 + 1) * E],
                    bp_sb[:], ALU.add,
                )
                eng = nc.sync if m % 2 == 0 else nc.scalar
                eng.dma_start(
                    out=out[m * 128:(m + 1) * 128, :], in_=ob[:]
                )

    return nc


def get_graph():
    global _GRAPH
    if _GRAPH is None:
        nc = _build_graph()
        nc.compile()
        _GRAPH = nc
    return _GRAPH


def make_in_maps(query, key_, value, Wq, bq, Wk, bk, Wv, bv, Wp, bp):
    query = np.asarray(query, np.float32)
    key_ = np.asarray(key_, np.float32)
    value = np.asarray(value, np.float32)
    Wq, Wk, Wv, Wp = (np.asarray(w, np.float32) for w in (Wq, Wk, Wv, Wp))
    bq, bk, bv, bp = (np.asarray(b_, np.float32) for b_ in (bq, bk, bv, bp))

    wq_b = np.ascontiguousarray(Wq).astype(BF)
    wk_b = np.ascontiguousarray(Wk).astype(BF)
    wv_b = np.ascontiguousarray(Wv).astype(BF)
    wp_b = np.ascontiguousarray(Wp).astype(BF)
    xt = {}
    for b in range(B):
        xt[("q", b)] = np.ascontiguousarray(query[:, b, :].T).astype(BF)
        xt[("k", b)] = np.ascontiguousarray(key_[:, b, :].T).astype(BF)
        xt[("v", b)] = np.ascontiguousarray(value[:, b, :].T).astype(BF)

    in_maps = []
    for c in range(NCORES):
        b = c // SP
        p = c % SP
        m = {
            "xqt": np.ascontiguousarray(xt[("q", b)][:, p * TQ:(p + 1) * TQ]),
            "xkt": xt[("k", b)],
            "xvt": xt[("v", b)],
            "wq": wq_b,
            "wk": wk_b,
            "wv": wv_b,
            "wp": wp_b,
            "bq": bq.reshape(1, E).copy(),
            "bk": bk.reshape(1, E).copy(),
            "bv": bv.reshape(1, E).copy(),
            "bp": bp.reshape(1, E).copy(),
        }
        in_maps.append(m)
    return in_maps


def assemble(results):
    out_full = np.empty((L, B, E), np.float32)
    for c in range(NCORES):
        b = c // SP
        p = c % SP
        out_full[p * TQ:(p + 1) * TQ, b, :] = results[c]["out"]
    return out_full


def run(inputs, trace=False, **kw):
    nc = get_graph()
    in_maps = make_in_maps(**inputs)
    res = run_bass_kernel_spmd(
        nc, in_maps, core_ids=list(range(NCORES)), trace=trace, **kw
    )
    return res


def kernel(**inputs):
    res = run(inputs, trace=False)
    return assemble(res.results)
